# revision 11
# baseline (speedup 1.0000x reference)
"""Exponential smoothing (linear recurrence scan) on 8 trn2 NeuronCores.

Math (per batch b, head h, dim d):
    alpha = sigmoid(smoothing_weight[h])
    u[t]  = (1-alpha)*values[t] + factor*alpha*aux_values[t]
    y[t]  = alpha*y[t-1] + u[t],   y[-1] = v0
Sharding: data-parallel over batch b -> 8 cores, one batch each.

Device algorithm (per core, T=4096, HD=H*D=512), fp32 data, fp32r matmuls:
  - T is split into NG groups x CPG chunks of CH=64 rows.
  - Main matmul per (head, group): K-stacked operands  rhs = [v_chunk; a_chunk]
    (64+64 rows), weights = [c1*L; c2*L]^T, so one pass computes
    u-scan directly:  Y_local[p] = sum_q L[p,q] u[q].  Chunks are batched
    along the free dim (float32r -> fast at N=512).
  - Chunk summaries s_c = Y_local[c][63] (row 63 of psA) are staged to a
    single SBUF row, gathered by one SBUF->SBUF DMA into [chunk, hd] layout.
  - Level-2 (augmented): one [17,17] x [17,64] matmul per (head, group)
    computes all carries P_c = S_{c-1} AND the group-exit state S_out, with
    the incoming carry appended as row 16 of the rhs.
  - Fixup: rank-1 matmul decay_h (x) P_row ACCUMULATED into the main psA
    (start=False), then a single copy evacuates the finished chunk.
"""

import sys

sys.path.insert(0, "/opt/trn_rl_repo")

import numpy as np

import concourse.bass as bass
import concourse.bacc as bacc
import concourse.mybir as mybir
from concourse.tile import TileContext
from concourse.bass_utils import run_bass_kernel_spmd

B, T, H, D = 8, 4096, 8, 64
HD = H * D                  # 512
CH = 64                     # chunk length (rows per chunk)
CPG = 16                    # chunks per group
NG = T // (CH * CPG)        # 4 groups
GT = CH * CPG               # 1024 rows per group
C2 = CPG // 2               # free-dim chunk pairs in y layout

F32 = mybir.dt.float32
F32R = mybir.dt.float32r


def build_consts(smoothing_weight, v0):
    """Host-side constant tensors (float64 math, cast to fp32)."""
    a = 1.0 / (1.0 + np.exp(-smoothing_weight.astype(np.float64).reshape(H)))
    c1 = 1.0 - a
    factor = c1 / np.maximum(c1, 1e-6)
    c2 = factor * a

    q = np.arange(CH)
    e = q[None, :] - q[:, None]                     # [q, p] -> p - q
    pow_ = np.where(e >= 0, a[:, None, None] ** np.maximum(e, 0), 0.0)  # [h,q,p]
    # K-stacked lhsT per head: [128, 64] = [c1*L^T (rows 0-63); c2*L^T (64-127)]
    wstack = np.concatenate(
        [c1[:, None, None] * pow_, c2[:, None, None] * pow_], axis=1
    )                                                # [h, 128, 64]
    wstack = wstack.transpose(1, 0, 2).reshape(2 * CH, H * CH)

    decay = (a[:, None] ** (q[None, :] + 1)).reshape(1, H * CH)  # [1, h*64]

    A = a ** CH                                      # alpha^64 per head
    r = np.arange(CPG + 1)
    # level-2 aug lhsT [17, 17]: rows j=0..15 -> A^(r-1-j) for j<=r-1;
    # row j=16 (incoming carry) -> A^r
    ee = (r[None, :] - 1) - np.arange(CPG)[:, None]  # [j, r]
    mex = np.where(ee >= 0, A[:, None, None] ** np.maximum(ee, 0), 0.0)  # [h,16,17]
    carry_row = (A[:, None] ** r[None, :])[:, None, :]               # [h,1,17]
    mex_aug = np.concatenate([mex, carry_row], axis=1)               # [h,17,17]
    mex_aug = mex_aug.transpose(1, 0, 2).reshape(CPG + 1, H * (CPG + 1))

    v0row = v0.astype(np.float64).reshape(1, HD)

    f = np.float32
    return {
        "wstack": np.ascontiguousarray(wstack, dtype=f),
        "decay": np.ascontiguousarray(decay, dtype=f),
        "mexc": np.ascontiguousarray(mex_aug, dtype=f),
        "v0r": np.ascontiguousarray(v0row, dtype=f),
    }


def build_nc():
    nc = bacc.Bacc()

    v_d = nc.declare_dram_parameter("v", [T, HD], F32R, isOutput=False)
    a_d = nc.declare_dram_parameter("a", [T, HD], F32R, isOutput=False)
    w_d = nc.declare_dram_parameter("wstack", [2 * CH, H * CH], F32R, isOutput=False)
    dec_d = nc.declare_dram_parameter("decay", [1, H * CH], F32R, isOutput=False)
    mex_d = nc.declare_dram_parameter("mexc", [CPG + 1, H * (CPG + 1)], F32,
                                      isOutput=False)
    v0_d = nc.declare_dram_parameter("v0r", [1, HD], F32, isOutput=False)
    y_d = nc.declare_dram_parameter("y", [T, HD], F32, isOutput=True)

    with TileContext(nc) as tc:
        with (
            tc.tile_pool(name="wpool", bufs=1) as wpool,
            tc.tile_pool(name="vain", bufs=2) as vain,
            tc.tile_pool(name="yout", bufs=2) as yout,
            tc.tile_pool(name="srows", bufs=1) as srpool,
            tc.tile_pool(name="small", bufs=2) as small,
            tc.tile_pool(name="prows", bufs=1) as prpool,
            tc.tile_pool(name="psA", bufs=3, space="PSUM") as psA_pool,
            tc.tile_pool(name="psP", bufs=2, space="PSUM") as psP_pool,
        ):
            # constants -> SBUF once
            w = wpool.tile([2 * CH, H * CH], F32R, tag="w")
            dec = wpool.tile([1, H * CH], F32R, tag="dec")
            mex = wpool.tile([CPG + 1, H * (CPG + 1)], F32, tag="mex")
            nc.sync.dma_start(w[:], w_d[:])
            nc.sync.dma_start(dec[:], dec_d[:])
            nc.sync.dma_start(mex[:], mex_d[:])

            sT_prev = None
            p_sb_prev = None

            for g in range(NG):
                r0 = g * GT
                # ---- stream group inputs, K-stacked: v -> rows 0-63, a -> 64-127
                va = vain.tile([2 * CH, CPG * HD], F32R, tag="va")
                va3 = va[:].rearrange("p (c m) -> p c m", c=CPG)
                src_v = v_d[r0:r0 + GT, :].rearrange("(c p) m -> p c m", c=CPG, p=CH)
                src_a = a_d[r0:r0 + GT, :].rearrange("(c p) m -> p c m", c=CPG, p=CH)
                nc.sync.dma_start(va3[0:CH, :, :], src_v)
                nc.sync.dma_start(va3[CH:2 * CH, :, :], src_a)

                y_sb = yout.tile([2 * CH, C2 * HD], F32, tag="y")
                y3 = y_sb[:].rearrange("p (c m) -> p c m", c=C2)

                # staging row for chunk summaries, free layout (c, h, d)
                srow = srpool.tile([CH, CPG * HD], F32, tag="srow")
                s3 = srow[:].rearrange("p (c h d) -> p c h d", c=CPG, h=H, d=D)

                # carry rhs for level-2: [17, 512]; row 16 = incoming state
                sT = small.tile([CPG + 1, HD], F32, tag="sT")
                if g == 0:
                    nc.sync.dma_start(sT[CPG:CPG + 1, :], v0_d[:])
                else:
                    # ACT partition start must be 32-aligned: copy the whole
                    # tile (rows 0-15 are overwritten by the gathers below)
                    nc.scalar.copy(sT[:], p_sb_prev[:])

                psP = psP_pool.tile([CPG + 1, HD], F32, tag="psP")
                p_sb = small.tile([CPG + 1, HD], F32, tag="p_sb")
                prow = prpool.tile([1, CPG * HD], F32, tag="prow")
                prow3 = prow[:].rearrange("o (c m) -> o c m", c=CPG)

                # head pairs: each pair has its own gather->lvl2->scatter->fixup
                # chain, so a pair's fixups depend only on its own mains and
                # the PSUM pool never deadlocks.
                for hp in range(H // 2):
                    pair = (2 * hp, 2 * hp + 1)
                    psAs = {}
                    for h in pair:
                        psA = psA_pool.tile([CH, CPG * D], F32, tag="psA")
                        psAs[h] = psA
                        rhs = va3[:, :, h * D:(h + 1) * D]    # [128, CPG, D]
                        lhsT = w[:, h * CH:(h + 1) * CH]      # [128, 64]
                        nc.tensor.matmul(psA[:, 0:512], lhsT, rhs[:, 0:C2, :],
                                         start=True, stop=False)
                        nc.tensor.matmul(psA[:, 512:1024], lhsT,
                                         rhs[:, C2:CPG, :],
                                         start=True, stop=False)
                        # stage chunk summaries (pre-carry row 63); PSUM reads
                        # must start 32-aligned, so copy rows 32-63
                        nc.scalar.copy(s3[CH - 32:CH, :, h, :],
                                       psA[CH - 32:CH, :].rearrange(
                                           "p (c d) -> p c d", c=CPG))

                    ps = slice(hp * 2 * D, (hp + 1) * 2 * D)   # pair free slice
                    # gather pair summaries: [1,(c, 2h*d)] -> sT rows
                    sflat = srow[:].rearrange("p (c m) -> p c m", c=CPG)
                    nc.gpsimd.dma_start(sT[0:CPG, ps],
                                        sflat[CH - 1:CH, :, ps])

                    # level-2 augmented matmuls for the pair
                    for h in pair:
                        hs = slice(h * (CPG + 1), (h + 1) * (CPG + 1))
                        nc.tensor.matmul(psP[:, h * D:(h + 1) * D], mex[:, hs],
                                         sT[:, h * D:(h + 1) * D],
                                         start=True, stop=True)
                    nc.scalar.copy(p_sb[:, ps], psP[:, ps])

                    # scatter pair carries into the prow row
                    nc.gpsimd.dma_start(
                        prow3[:, :, 2 * hp * D:(2 * hp + 2) * D],
                        p_sb[0:CPG, ps])

                    # fixup accumulated into psA, then evacuate
                    for h in pair:
                        psA = psAs[h]
                        dh = dec[0:1, h * CH:(h + 1) * CH]    # [1, 64]
                        rp = prow3[:, :, h * D:(h + 1) * D].bitcast(F32R)
                        nc.tensor.matmul(psA[:, 0:512], dh, rp[:, 0:C2, :],
                                         start=False, stop=True)
                        nc.tensor.matmul(psA[:, 512:1024], dh,
                                         rp[:, C2:CPG, :],
                                         start=False, stop=True)
                        # evacuate: even chunks -> parts 0-63, odd -> 64-127
                        pc = psA[:].rearrange("p (c d) -> p c d", c=CPG)
                        nc.scalar.copy(y3[0:CH, :, h * D:(h + 1) * D],
                                       pc[:, 0::2, :])
                        nc.scalar.copy(y3[CH:2 * CH, :, h * D:(h + 1) * D],
                                       pc[:, 1::2, :])

                # ---- store group output
                dst = y_d[r0:r0 + GT, :].rearrange("(c p) m -> p c m", c=C2, p=2 * CH)
                nc.scalar.dma_start(dst, y_sb[:].rearrange("p (c m) -> p c m", c=C2))

                p_sb_prev = p_sb

    nc.finalize()
    return nc


_NC_CACHE = None


def _get_nc():
    global _NC_CACHE
    if _NC_CACHE is None:
        _NC_CACHE = build_nc()
    return _NC_CACHE


def kernel(values, aux_values, v0, smoothing_weight):
    consts = build_consts(smoothing_weight, v0)
    nc = _get_nc()
    in_maps = []
    for b in range(B):
        m = dict(consts)
        m["v"] = np.ascontiguousarray(values[b].reshape(T, HD), dtype=np.float32)
        m["a"] = np.ascontiguousarray(aux_values[b].reshape(T, HD), dtype=np.float32)
        in_maps.append(m)
    res = run_bass_kernel_spmd(nc, in_maps, list(range(B))).results
    out = np.stack([res[b]["y"].reshape(T, H, D) for b in range(B)])
    return out.astype(np.float32)


# revision 12
# speedup vs baseline: 1.6326x; 1.6326x over previous
"""Exponential smoothing (linear recurrence scan) on 8 trn2 NeuronCores.

Math (per batch b, head h, dim d):
    alpha = sigmoid(smoothing_weight[h])
    u[t]  = (1-alpha)*values[t] + factor*alpha*aux_values[t]
    y[t]  = alpha*y[t-1] + u[t],   y[-1] = v0
Sharding: data-parallel over batch b -> 8 cores, one batch each.

Device algorithm (per core, T=4096, HD=H*D=512), fp32 data, fp32r matmuls:
  - T in NG groups x CPG chunks of CH=64 rows.  SBUF y layout: partition =
    t mod 128 (= 64*(c%2) + p), free = (c//2, h, d).
  - Main matmuls per (head, group): K-stacked rhs [v_chunk; a_chunk]
    (64+64 rows) x weights [c1*L; c2*L]^T compute the u-scan in one pass.
    Even chunks -> psA[:, 0:512], odd chunks -> psA[:, 512:1024]; evacuated
    contiguously to the two partition halves of y_sb.
  - Chunk summaries = y_sb rows 63 (even chunks) / 127 (odd) -> two gather
    DMAs into sT rows (sigma order: evens then odds); row 16 = carry-in.
  - Level-2: one augmented [17,17] matmul per head computes all chunk
    carries + the group-exit state (host-permuted power matrix).
  - Fixup: K=2 matmul (decay-even/decay-odd rows) produces the carry
    contribution for BOTH partition halves at once -> psB [128, 512];
    a DVE add folds it into y_sb.
"""

import sys

sys.path.insert(0, "/opt/trn_rl_repo")

import numpy as np

import concourse.bass as bass
import concourse.bacc as bacc
import concourse.mybir as mybir
from concourse.tile import TileContext
from concourse.bass_utils import run_bass_kernel_spmd

B, T, H, D = 8, 4096, 8, 64
HD = H * D                  # 512
CH = 64                     # chunk length
CPG = 16                    # chunks per group
NG = T // (CH * CPG)        # 4 groups
GT = CH * CPG               # 1024 rows per group
C2 = CPG // 2               # chunk pairs per group (free dim of y)

F32 = mybir.dt.float32
F32R = mybir.dt.float32r

# sigma: sT/psP row order = even chunks, then odd chunks, then exit state
SIGMA = list(range(0, CPG, 2)) + list(range(1, CPG, 2)) + [CPG]


def build_consts(smoothing_weight, v0):
    """Host-side constant tensors (float64 math, cast to fp32)."""
    a = 1.0 / (1.0 + np.exp(-smoothing_weight.astype(np.float64).reshape(H)))
    c1 = 1.0 - a
    factor = c1 / np.maximum(c1, 1e-6)
    c2 = factor * a

    q = np.arange(CH)
    e = q[None, :] - q[:, None]                     # [q, p] -> p - q
    pow_ = np.where(e >= 0, a[:, None, None] ** np.maximum(e, 0), 0.0)  # [h,q,p]
    wstack = np.concatenate(
        [c1[:, None, None] * pow_, c2[:, None, None] * pow_], axis=1
    ).transpose(1, 0, 2).reshape(2 * CH, H * CH)

    decay = a[:, None] ** (q[None, :] + 1)          # [h, 64]
    deceo = np.zeros((H, 2, 2 * CH))
    deceo[:, 0, 0:CH] = decay
    deceo[:, 1, CH:2 * CH] = decay
    deceo = deceo.transpose(1, 0, 2).reshape(2, H * 2 * CH)

    A = a ** CH                                      # alpha^64 per head
    # augmented level-2 lhsT in sigma order: rows j' (contraction, = sT rows),
    # cols r' (outputs, = psP rows).  P_{c} = A^c S_in + sum_{j<c} A^(c-1-j) s_j
    n = CPG + 1
    mex = np.zeros((H, n, n))
    for jp in range(n):
        for rp in range(n):
            sj, sr = SIGMA[jp], SIGMA[rp]
            if jp == CPG:                            # carry-in row
                mex[:, jp, rp] = A ** sr
            elif sj <= sr - 1:
                mex[:, jp, rp] = A ** (sr - 1 - sj)
    mex = mex.transpose(1, 0, 2).reshape(n, H * n)

    v0row = v0.astype(np.float64).reshape(1, HD)

    f = np.float32
    return {
        "wstack": np.ascontiguousarray(wstack, dtype=f),
        "deceo": np.ascontiguousarray(deceo, dtype=f),
        "mexc": np.ascontiguousarray(mex, dtype=f),
        "v0r": np.ascontiguousarray(v0row, dtype=f),
    }


def build_nc():
    nc = bacc.Bacc()

    v_d = nc.declare_dram_parameter("v", [T, HD], F32R, isOutput=False)
    a_d = nc.declare_dram_parameter("a", [T, HD], F32R, isOutput=False)
    w_d = nc.declare_dram_parameter("wstack", [2 * CH, H * CH], F32R,
                                    isOutput=False)
    dec_d = nc.declare_dram_parameter("deceo", [2, H * 2 * CH], F32R,
                                      isOutput=False)
    mex_d = nc.declare_dram_parameter("mexc", [CPG + 1, H * (CPG + 1)], F32,
                                      isOutput=False)
    v0_d = nc.declare_dram_parameter("v0r", [1, HD], F32, isOutput=False)
    y_d = nc.declare_dram_parameter("y", [T, HD], F32, isOutput=True)

    with TileContext(nc) as tc:
        with (
            tc.tile_pool(name="wpool", bufs=1) as wpool,
            tc.tile_pool(name="vain", bufs=3) as vain,
            tc.tile_pool(name="yout", bufs=2) as yout,
            tc.tile_pool(name="small", bufs=2) as small,
            tc.tile_pool(name="psA", bufs=2, space="PSUM") as psA_pool,
            tc.tile_pool(name="psP", bufs=2, space="PSUM") as psP_pool,
            tc.tile_pool(name="psB", bufs=2, space="PSUM") as psB_pool,
        ):
            w = wpool.tile([2 * CH, H * CH], F32R, tag="w")
            dec = wpool.tile([2, H * 2 * CH], F32R, tag="dec")
            mex = wpool.tile([CPG + 1, H * (CPG + 1)], F32, tag="mex")
            nc.sync.dma_start(w[:], w_d[:])
            nc.sync.dma_start(dec[:], dec_d[:])
            nc.sync.dma_start(mex[:], mex_d[:])

            p_sb_prev = None

            for g in range(NG):
                r0 = g * GT
                va = vain.tile([2 * CH, CPG * HD], F32R, tag="va")
                va3 = va[:].rearrange("p (c m) -> p c m", c=CPG)
                src_v = v_d[r0:r0 + GT, :].rearrange("(c p) m -> p c m",
                                                     c=CPG, p=CH)
                src_a = a_d[r0:r0 + GT, :].rearrange("(c p) m -> p c m",
                                                     c=CPG, p=CH)
                nc.sync.dma_start(va3[0:CH, :, :], src_v)
                nc.sync.dma_start(va3[CH:2 * CH, :, :], src_a)

                y_sb = yout.tile([2 * CH, C2 * HD], F32, tag="y")
                y3 = y_sb[:].rearrange("p (c m) -> p c m", c=C2)

                sT = small.tile([CPG + 1, HD], F32, tag="sT")
                if g == 0:
                    nc.gpsimd.dma_start(sT[CPG:CPG + 1, :], v0_d[:])
                else:
                    nc.gpsimd.dma_start(sT[CPG:CPG + 1, :],
                                        p_sb_prev[CPG:CPG + 1, :])

                # ---- main K-stacked scan matmuls + immediate evacuation
                for h in range(H):
                    psA = psA_pool.tile([CH, CPG * D], F32, tag="psA")
                    lhsT = w[:, h * CH:(h + 1) * CH]          # [128, 64]
                    rhs = va3[:, :, h * D:(h + 1) * D]        # [128, CPG, D]
                    nc.tensor.matmul(psA[:, 0:512], lhsT, rhs[:, 0::2, :],
                                     start=True, stop=True)
                    nc.tensor.matmul(psA[:, 512:1024], lhsT, rhs[:, 1::2, :],
                                     start=True, stop=True)
                    pe = psA[:, 0:512].rearrange("p (c d) -> p c d", c=C2)
                    po = psA[:, 512:1024].rearrange("p (c d) -> p c d", c=C2)
                    nc.scalar.copy(y3[0:CH, :, h * D:(h + 1) * D], pe)
                    nc.scalar.copy(y3[CH:2 * CH, :, h * D:(h + 1) * D], po)

                # ---- gather chunk summaries (rows 63 / 127 of y_sb)
                nc.gpsimd.dma_start(sT[0:C2, :], y_sb[CH - 1:CH, :])
                nc.gpsimd.dma_start(sT[C2:CPG, :], y_sb[2 * CH - 1:2 * CH, :])

                # ---- level-2: one augmented matmul per head
                psP = psP_pool.tile([CPG + 1, HD], F32, tag="psP")
                for h in range(H):
                    hs = slice(h * (CPG + 1), (h + 1) * (CPG + 1))
                    nc.tensor.matmul(psP[:, h * D:(h + 1) * D], mex[:, hs],
                                     sT[:, h * D:(h + 1) * D],
                                     start=True, stop=True)
                p_sb = small.tile([CPG + 1, HD], F32, tag="p_sb")
                nc.scalar.copy(p_sb[:], psP[:])

                # ---- scatter carries: rows 0-7 -> partition 0 (even),
                #      rows 8-15 -> partition 1 (odd)
                prow = small.tile([2, C2 * HD], F32, tag="prow")
                nc.gpsimd.dma_start(prow[:], p_sb[0:CPG, :])
                prow3 = prow[:].rearrange("p (c m) -> p c m", c=C2)

                # ---- fixup: K=2 rank-2 matmul -> psB, DVE-added into y_sb
                for h in range(H):
                    psB = psB_pool.tile([2 * CH, C2 * D], F32, tag="psB")
                    rp = prow3[:, :, h * D:(h + 1) * D].bitcast(F32R)
                    nc.tensor.matmul(psB[:], dec[:, h * 2 * CH:(h + 1) * 2 * CH],
                                     rp, start=True, stop=True)
                    yv = y3[:, :, h * D:(h + 1) * D]
                    nc.vector.tensor_add(
                        yv, yv, psB[:].rearrange("p (c d) -> p c d", c=C2))

                # ---- store group output
                dst = y_d[r0:r0 + GT, :].rearrange("(c p) m -> p c m",
                                                   c=C2, p=2 * CH)
                nc.scalar.dma_start(dst, y_sb[:].rearrange("p (c m) -> p c m",
                                                           c=C2))

                p_sb_prev = p_sb

    nc.finalize()
    return nc


_NC_CACHE = None


def _get_nc():
    global _NC_CACHE
    if _NC_CACHE is None:
        _NC_CACHE = build_nc()
    return _NC_CACHE


def kernel(values, aux_values, v0, smoothing_weight):
    consts = build_consts(smoothing_weight, v0)
    nc = _get_nc()
    in_maps = []
    for b in range(B):
        m = dict(consts)
        m["v"] = np.ascontiguousarray(values[b].reshape(T, HD), dtype=np.float32)
        m["a"] = np.ascontiguousarray(aux_values[b].reshape(T, HD), dtype=np.float32)
        in_maps.append(m)
    res = run_bass_kernel_spmd(nc, in_maps, list(range(B))).results
    out = np.stack([res[b]["y"].reshape(T, H, D) for b in range(B)])
    return out.astype(np.float32)


# revision 16
# speedup vs baseline: 1.6606x; 1.0171x over previous
"""Exponential smoothing (linear recurrence scan) on 8 trn2 NeuronCores.

Math (per batch b, head h, dim d):
    alpha = sigmoid(smoothing_weight[h])
    u[t]  = (1-alpha)*values[t] + factor*alpha*aux_values[t]
    y[t]  = alpha*y[t-1] + u[t],   y[-1] = v0
Sharding: data-parallel over batch b -> 8 cores, one batch each.

Device algorithm (per core, T=4096, HD=H*D=512), fp32 data, fp32r matmuls:
  - T in NG groups x CPG chunks of CH=64 rows.  SBUF y layout: partition =
    t mod 128 (= 64*(c%2) + p), free = (c//2, h, d).
  - Main matmuls per (head, group): K-stacked rhs [v_chunk; a_chunk]
    (64+64 rows) x weights [c1*L; c2*L]^T compute the u-scan in one pass.
    Even chunks -> psA[:, 0:512], odd chunks -> psA[:, 512:1024]; evacuated
    contiguously to the two partition halves of y_sb.
  - Chunk summaries = y_sb rows 63 (even chunks) / 127 (odd) -> two gather
    DMAs into sT rows (sigma order: evens then odds); row 16 = carry-in.
  - Level-2: one augmented [17,17] matmul per head computes all chunk
    carries + the group-exit state (host-permuted power matrix).
  - Fixup: K=2 matmul (decay-even/decay-odd rows) produces the carry
    contribution for BOTH partition halves at once -> psB [128, 512];
    a DVE add folds it into y_sb.
"""

import sys

sys.path.insert(0, "/opt/trn_rl_repo")

import numpy as np

import concourse.bass as bass
import concourse.bacc as bacc
import concourse.mybir as mybir
from concourse.tile import TileContext
from concourse.bass_utils import run_bass_kernel_spmd

B, T, H, D = 8, 4096, 8, 64
HD = H * D                  # 512
CH = 64                     # chunk length
CPG = 16                    # chunks per group
NG = T // (CH * CPG)        # 4 groups
GT = CH * CPG               # 1024 rows per group
C2 = CPG // 2               # chunk pairs per group (free dim of y)

F32 = mybir.dt.float32
F32R = mybir.dt.float32r

# sigma: sT/psP row order = even chunks, then odd chunks, then exit state
SIGMA = list(range(0, CPG, 2)) + list(range(1, CPG, 2)) + [CPG]


def build_consts(smoothing_weight, v0):
    """Host-side constant tensors (float64 math, cast to fp32)."""
    a = 1.0 / (1.0 + np.exp(-smoothing_weight.astype(np.float64).reshape(H)))
    c1 = 1.0 - a
    factor = c1 / np.maximum(c1, 1e-6)
    c2 = factor * a

    q = np.arange(CH)
    e = q[None, :] - q[:, None]                     # [q, p] -> p - q
    pow_ = np.where(e >= 0, a[:, None, None] ** np.maximum(e, 0), 0.0)  # [h,q,p]
    wstack = np.concatenate(
        [c1[:, None, None] * pow_, c2[:, None, None] * pow_], axis=1
    ).transpose(1, 0, 2).reshape(2 * CH, H * CH)

    decay = a[:, None] ** (q[None, :] + 1)          # [h, 64]
    deceo = np.zeros((H, 2, 2 * CH))
    deceo[:, 0, 0:CH] = decay
    deceo[:, 1, CH:2 * CH] = decay
    deceo = deceo.transpose(1, 0, 2).reshape(2, H * 2 * CH)

    A = a ** CH                                      # alpha^64 per head
    # augmented level-2 lhsT in sigma order: rows j' (contraction, = sT rows),
    # cols r' (outputs, = psP rows).  P_{c} = A^c S_in + sum_{j<c} A^(c-1-j) s_j
    n = CPG + 1
    mex = np.zeros((H, n, n))
    for jp in range(n):
        for rp in range(n):
            sj, sr = SIGMA[jp], SIGMA[rp]
            if jp == CPG:                            # carry-in row
                mex[:, jp, rp] = A ** sr
            elif sj <= sr - 1:
                mex[:, jp, rp] = A ** (sr - 1 - sj)
    mex = mex.transpose(1, 0, 2).reshape(n, H * n)

    v0row = v0.astype(np.float64).reshape(1, HD)

    f = np.float32
    return {
        "wstack": np.ascontiguousarray(wstack, dtype=f),
        "deceo": np.ascontiguousarray(deceo, dtype=f),
        "mexc": np.ascontiguousarray(mex, dtype=f),
        "v0r": np.ascontiguousarray(v0row, dtype=f),
    }


def build_nc():
    nc = bacc.Bacc()

    v_d = nc.declare_dram_parameter("v", [T, HD], F32R, isOutput=False)
    a_d = nc.declare_dram_parameter("a", [T, HD], F32R, isOutput=False)
    w_d = nc.declare_dram_parameter("wstack", [2 * CH, H * CH], F32R,
                                    isOutput=False)
    dec_d = nc.declare_dram_parameter("deceo", [2, H * 2 * CH], F32R,
                                      isOutput=False)
    mex_d = nc.declare_dram_parameter("mexc", [CPG + 1, H * (CPG + 1)], F32R,
                                      isOutput=False)
    v0_d = nc.declare_dram_parameter("v0r", [1, HD], F32R, isOutput=False)
    y_d = nc.declare_dram_parameter("y", [T, HD], F32, isOutput=True)

    with TileContext(nc) as tc:
        with (
            tc.tile_pool(name="wpool", bufs=1) as wpool,
            tc.tile_pool(name="vain", bufs=3) as vain,
            tc.tile_pool(name="yout", bufs=2) as yout,
            tc.tile_pool(name="small", bufs=2) as small,
            tc.tile_pool(name="psA", bufs=2, space="PSUM") as psA_pool,
            tc.tile_pool(name="psP", bufs=2, space="PSUM") as psP_pool,
            tc.tile_pool(name="psB", bufs=2, space="PSUM") as psB_pool,
        ):
            w = wpool.tile([2 * CH, H * CH], F32R, tag="w")
            dec = wpool.tile([2, H * 2 * CH], F32R, tag="dec")
            mex = wpool.tile([CPG + 1, H * (CPG + 1)], F32R, tag="mex")
            nc.scalar.dma_start(w[:], w_d[:])
            nc.scalar.dma_start(dec[:], dec_d[:])
            nc.scalar.dma_start(mex[:], mex_d[:])

            p_sb_prev = None

            for g in range(NG):
                r0 = g * GT
                va = vain.tile([2 * CH, CPG * HD], F32R, tag="va")
                va3 = va[:].rearrange("p (c m) -> p c m", c=CPG)
                src_v = v_d[r0:r0 + GT, :].rearrange("(c p) m -> p c m",
                                                     c=CPG, p=CH)
                src_a = a_d[r0:r0 + GT, :].rearrange("(c p) m -> p c m",
                                                     c=CPG, p=CH)
                nc.sync.dma_start(va3[0:CH, :, :], src_v)
                nc.sync.dma_start(va3[CH:2 * CH, :, :], src_a)

                y_sb = yout.tile([2 * CH, C2 * HD], F32, tag="y")
                y3 = y_sb[:].rearrange("p (c m) -> p c m", c=C2)

                sT = small.tile([CPG + 1, HD], F32, tag="sT")
                if g == 0:
                    nc.gpsimd.dma_start(sT[CPG:CPG + 1, :].bitcast(F32R), v0_d[:])
                else:
                    nc.gpsimd.dma_start(sT[CPG:CPG + 1, :],
                                        p_sb_prev[CPG:CPG + 1, :])

                # ---- main K-stacked scan matmuls + immediate evacuation
                for h in range(H):
                    psA = psA_pool.tile([CH, CPG * D], F32, tag="psA")
                    lhsT = w[:, h * CH:(h + 1) * CH]          # [128, 64]
                    rhs = va3[:, :, h * D:(h + 1) * D]        # [128, CPG, D]
                    nc.tensor.matmul(psA[:, 0:512], lhsT, rhs[:, 0::2, :],
                                     start=True, stop=True)
                    nc.tensor.matmul(psA[:, 512:1024], lhsT, rhs[:, 1::2, :],
                                     start=True, stop=True)
                    pe = psA[:, 0:512].rearrange("p (c d) -> p c d", c=C2)
                    po = psA[:, 512:1024].rearrange("p (c d) -> p c d", c=C2)
                    nc.scalar.copy(y3[0:CH, :, h * D:(h + 1) * D], pe)
                    nc.scalar.copy(y3[CH:2 * CH, :, h * D:(h + 1) * D], po)

                # ---- gather chunk summaries (rows 63 / 127 of y_sb)
                nc.gpsimd.dma_start(sT[0:C2, :], y_sb[CH - 1:CH, :])
                nc.gpsimd.dma_start(sT[C2:CPG, :], y_sb[2 * CH - 1:2 * CH, :])

                # ---- level-2: one augmented matmul per head
                psP = psP_pool.tile([CPG + 1, HD], F32, tag="psP")
                for h in range(H):
                    hs = slice(h * (CPG + 1), (h + 1) * (CPG + 1))
                    nc.tensor.matmul(psP[:, h * D:(h + 1) * D], mex[:, hs],
                                     sT[:, h * D:(h + 1) * D].bitcast(F32R),
                                     start=True, stop=True)
                p_sb = small.tile([CPG + 1, HD], F32, tag="p_sb")
                nc.scalar.copy(p_sb[:], psP[:])

                # ---- scatter carries: rows 0-7 -> partition 0 (even),
                #      rows 8-15 -> partition 1 (odd)
                prow = small.tile([2, C2 * HD], F32, tag="prow")
                nc.gpsimd.dma_start(prow[:], p_sb[0:CPG, :])
                prow3 = prow[:].rearrange("p (c m) -> p c m", c=C2)

                # ---- fixup: K=2 rank-2 matmul -> psB, DVE-added into y_sb
                for h in range(H):
                    psB = psB_pool.tile([2 * CH, C2 * D], F32, tag="psB")
                    rp = prow3[:, :, h * D:(h + 1) * D].bitcast(F32R)
                    nc.tensor.matmul(psB[:], dec[:, h * 2 * CH:(h + 1) * 2 * CH],
                                     rp, start=True, stop=True)
                    yv = y3[:, :, h * D:(h + 1) * D]
                    nc.vector.tensor_add(
                        yv, yv, psB[:].rearrange("p (c d) -> p c d", c=C2))

                # ---- store group output
                dst = y_d[r0:r0 + GT, :].rearrange("(c p) m -> p c m",
                                                   c=C2, p=2 * CH)
                nc.scalar.dma_start(dst, y_sb[:].rearrange("p (c m) -> p c m",
                                                           c=C2))

                p_sb_prev = p_sb

    nc.finalize()
    return nc


_NC_CACHE = None


def _get_nc():
    global _NC_CACHE
    if _NC_CACHE is None:
        _NC_CACHE = build_nc()
    return _NC_CACHE


def kernel(values, aux_values, v0, smoothing_weight):
    consts = build_consts(smoothing_weight, v0)
    nc = _get_nc()
    in_maps = []
    for b in range(B):
        m = dict(consts)
        m["v"] = np.ascontiguousarray(values[b].reshape(T, HD), dtype=np.float32)
        m["a"] = np.ascontiguousarray(aux_values[b].reshape(T, HD), dtype=np.float32)
        in_maps.append(m)
    res = run_bass_kernel_spmd(nc, in_maps, list(range(B))).results
    out = np.stack([res[b]["y"].reshape(T, H, D) for b in range(B)])
    return out.astype(np.float32)


# revision 17
# speedup vs baseline: 2.4150x; 1.4543x over previous
"""Exponential smoothing (linear recurrence scan) on 8 trn2 NeuronCores.

Math (per batch b, head h, dim d):
    alpha = sigmoid(smoothing_weight[h])
    u[t]  = (1-alpha)*values[t] + factor*alpha*aux_values[t]
    y[t]  = alpha*y[t-1] + u[t],   y[-1] = v0
Sharding: data-parallel over batch b -> 8 cores, one batch each.

Device algorithm (per core, T=4096, HD=H*D=512), all fp32 data:
  - T is split into NG groups x CPG chunks of 128 rows.
  - Main: per head, Y_local = (c1*L)^T-matmul(v) + (c2*L)^T-matmul(a) where
    L[p,q] = alpha^(p-q) (p>=q) is the within-chunk scan matrix.  Chunks are
    batched along the matmul free dim (float32r -> 1 cycle/row at N>=256).
  - Level-2: chunk summaries s_c = Y_local[c][127] are scanned across chunks
    with small per-head matmuls using A = alpha^128 power matrices, yielding
    the carry P_c = S_{c-1} entering each chunk (S_{-1} = v0).
  - Fixup: rank-1 matmul decay_h (x) P_row accumulated into the output:
    y[128c+p] = Y_local[c][p] + alpha^(p+1) * P_c.
Cross-partition moves (chunk summaries -> partition-per-chunk, carries ->
single row) are done with small SBUF->SBUF DMAs.
"""

import sys

sys.path.insert(0, "/opt/trn_rl_repo")

import numpy as np

import concourse.bass as bass
import concourse.bacc as bacc
import concourse.mybir as mybir
from concourse.tile import TileContext
from concourse.bass_utils import run_bass_kernel_spmd

B, T, H, D = 8, 4096, 8, 64
HD = H * D                  # 512
P = 128                     # chunk length / partitions
NCHUNK = T // P             # 32
CPG = 8                     # chunks per group
NG = NCHUNK // CPG          # 4 groups
GT = CPG * P                # 1024 rows per group

F32 = mybir.dt.float32
F32R = mybir.dt.float32r


def build_consts(smoothing_weight, v0):
    """Host-side constant tensors (float64 math, cast to fp32)."""
    a = 1.0 / (1.0 + np.exp(-smoothing_weight.astype(np.float64).reshape(H)))
    c1 = 1.0 - a
    factor = c1 / np.maximum(c1, 1e-6)
    c2 = factor * a

    q = np.arange(P)
    e = q[None, :] - q[:, None]                     # [q, p] -> p - q
    pow_ = np.where(e >= 0, a[:, None, None] ** np.maximum(e, 0), 0.0)  # [h,q,p]
    w1 = (c1[:, None, None] * pow_).transpose(1, 0, 2).reshape(P, H * P)
    w2 = (c2[:, None, None] * pow_).transpose(1, 0, 2).reshape(P, H * P)

    decay = (a[:, None] ** (q[None, :] + 1)).reshape(1, H * P)  # [1, h*128]

    A = a ** P                                       # alpha^128 per head
    r = np.arange(CPG + 1)
    # augmented lhsT [9,9]: rows j=0..7 -> A^(r-1-j) for j<=r-1; row 8 (carry) -> A^r
    ee = (r[None, :] - 1) - np.arange(CPG)[:, None]  # [j, r]
    mexc = np.where(ee >= 0, A[:, None, None] ** np.maximum(ee, 0), 0.0)  # [h,8,9]
    crow = (A[:, None] ** r[None, :])[:, None, :]                         # [h,1,9]
    mexc = np.concatenate([mexc, crow], axis=1)                           # [h,9,9]
    mexc = mexc.transpose(1, 0, 2).reshape(CPG + 1, H * (CPG + 1))

    v0row = v0.astype(np.float64).reshape(1, HD)

    f = np.float32
    return {
        "w1": np.ascontiguousarray(w1, dtype=f),
        "w2": np.ascontiguousarray(w2, dtype=f),
        "decay": np.ascontiguousarray(decay, dtype=f),
        "mexc": np.ascontiguousarray(mexc, dtype=f),
        "v0r": np.ascontiguousarray(v0row, dtype=f),
    }


def build_nc():
    nc = bacc.Bacc()

    v_d = nc.declare_dram_parameter("v", [T, HD], F32R, isOutput=False)
    a_d = nc.declare_dram_parameter("a", [T, HD], F32R, isOutput=False)
    w1_d = nc.declare_dram_parameter("w1", [P, H * P], F32R, isOutput=False)
    w2_d = nc.declare_dram_parameter("w2", [P, H * P], F32R, isOutput=False)
    dec_d = nc.declare_dram_parameter("decay", [1, H * P], F32R, isOutput=False)
    mex_d = nc.declare_dram_parameter("mexc", [CPG + 1, H * (CPG + 1)], F32R,
                                      isOutput=False)
    v0_d = nc.declare_dram_parameter("v0r", [1, HD], F32R, isOutput=False)
    y_d = nc.declare_dram_parameter("y", [T, HD], F32, isOutput=True)

    with TileContext(nc) as tc:
        with (
            tc.tile_pool(name="wpool", bufs=1) as wpool,
            tc.tile_pool(name="vin", bufs=2) as vin,
            tc.tile_pool(name="ain", bufs=2) as ain,
            tc.tile_pool(name="yout", bufs=2) as yout,
            tc.tile_pool(name="small", bufs=2) as small,
            tc.tile_pool(name="psA", bufs=4, space="PSUM") as psA_pool,
            tc.tile_pool(name="psP", bufs=2, space="PSUM") as psP_pool,
            tc.tile_pool(name="psB", bufs=2, space="PSUM") as psB_pool,
        ):
            # constants -> SBUF once
            w1 = wpool.tile([P, H * P], F32R, tag="w1")
            w2 = wpool.tile([P, H * P], F32R, tag="w2")
            dec = wpool.tile([1, H * P], F32R, tag="dec")
            mex = wpool.tile([CPG + 1, H * (CPG + 1)], F32R, tag="mex")
            nc.scalar.dma_start(w1[:], w1_d[:])
            nc.scalar.dma_start(w2[:], w2_d[:])
            nc.scalar.dma_start(dec[:], dec_d[:])
            nc.scalar.dma_start(mex[:], mex_d[:])

            p_sb_prev = None

            for g in range(NG):
                r0 = g * GT
                # ---- stream group inputs:  [GT, HD] -> [P, CPG, HD]
                v_sb = vin.tile([P, CPG * HD], F32R, tag="v")
                a_sb = ain.tile([P, CPG * HD], F32R, tag="a")
                src_v = v_d[r0:r0 + GT, :].rearrange("(c p) m -> p c m", c=CPG, p=P)
                src_a = a_d[r0:r0 + GT, :].rearrange("(c p) m -> p c m", c=CPG, p=P)
                nc.sync.dma_start(v_sb[:].rearrange("p (c m) -> p c m", c=CPG), src_v)
                nc.sync.dma_start(a_sb[:].rearrange("p (c m) -> p c m", c=CPG), src_a)

                y_sb = yout.tile([P, CPG * HD], F32, tag="y")
                v3 = v_sb[:].rearrange("p (c m) -> p c m", c=CPG)
                a3 = a_sb[:].rearrange("p (c m) -> p c m", c=CPG)
                y3 = y_sb[:].rearrange("p (c m) -> p c m", c=CPG)

                # ---- main within-chunk scan, per head
                for h in range(H):
                    psA = psA_pool.tile([P, CPG * D], F32, tag="psA")
                    rhs_v = v3[:, :, h * D:(h + 1) * D]   # [P, CPG, D]
                    rhs_a = a3[:, :, h * D:(h + 1) * D]
                    nc.tensor.matmul(psA[:], w1[:, h * P:(h + 1) * P], rhs_v,
                                     start=True, stop=False)
                    nc.tensor.matmul(psA[:], w2[:, h * P:(h + 1) * P], rhs_a,
                                     start=False, stop=True)
                    # evacuate into interleaved (c, h, d) layout
                    nc.scalar.copy(y3[:, :, h * D:(h + 1) * D],
                                   psA[:].rearrange("p (c d) -> p c d", c=CPG))

                # ---- gather chunk summaries: row 127 -> [CPG, HD];
                #      row 8 = incoming carry state
                sT = small.tile([CPG + 1, HD], F32, tag="sT")
                nc.gpsimd.dma_start(sT[0:CPG, :], y_sb[P - 1:P, :])
                if g == 0:
                    nc.gpsimd.dma_start(sT[CPG:CPG + 1, :].bitcast(F32R),
                                        v0_d[:])
                else:
                    nc.gpsimd.dma_start(sT[CPG:CPG + 1, :],
                                        p_sb_prev[CPG:CPG + 1, :])

                # ---- level-2 scan: one augmented F32R matmul per head
                psP = psP_pool.tile([CPG + 1, HD], F32, tag="psP")
                for h in range(H):
                    hs = slice(h * (CPG + 1), (h + 1) * (CPG + 1))
                    nc.tensor.matmul(psP[:, h * D:(h + 1) * D], mex[:, hs],
                                     sT[:, h * D:(h + 1) * D].bitcast(F32R),
                                     start=True, stop=True)
                p_sb = small.tile([CPG + 1, HD], F32, tag="p_sb")
                nc.scalar.copy(p_sb[:], psP[:])

                # ---- scatter carries to a single row [1, (c, h, d)]
                prow = small.tile([1, CPG * HD], F32, tag="prow")
                nc.gpsimd.dma_start(prow[:], p_sb[0:CPG, :])

                # ---- fixup: rank-1 decay (x) carry, then add into y
                prow3 = prow[:].rearrange("o (c m) -> o c m", c=CPG)
                for h in range(H):
                    psB = psB_pool.tile([P, CPG * D], F32, tag="psB")
                    nc.tensor.matmul(psB[:], dec[0:1, h * P:(h + 1) * P],
                                     prow3[:, :, h * D:(h + 1) * D].bitcast(F32R),
                                     start=True, stop=True)
                    yv = y3[:, :, h * D:(h + 1) * D]
                    nc.vector.tensor_add(
                        yv, yv, psB[:].rearrange("p (c d) -> p c d", c=CPG))

                # ---- store group output
                dst = y_d[r0:r0 + GT, :].rearrange("(c p) m -> p c m", c=CPG, p=P)
                nc.scalar.dma_start(dst, y_sb[:].rearrange("p (c m) -> p c m", c=CPG))

                p_sb_prev = p_sb

    nc.finalize()
    return nc


_NC_CACHE = None


def _get_nc():
    global _NC_CACHE
    if _NC_CACHE is None:
        _NC_CACHE = build_nc()
    return _NC_CACHE


def kernel(values, aux_values, v0, smoothing_weight):
    consts = build_consts(smoothing_weight, v0)
    nc = _get_nc()
    in_maps = []
    for b in range(B):
        m = dict(consts)
        m["v"] = np.ascontiguousarray(values[b].reshape(T, HD), dtype=np.float32)
        m["a"] = np.ascontiguousarray(aux_values[b].reshape(T, HD), dtype=np.float32)
        in_maps.append(m)
    res = run_bass_kernel_spmd(nc, in_maps, list(range(B))).results
    out = np.stack([res[b]["y"].reshape(T, H, D) for b in range(B)])
    return out.astype(np.float32)
